# revision 14
# baseline (speedup 1.0000x reference)
"""CMC-V2 loss kernel for 8 Trainium2 NeuronCores (Bass/Tile).

Math
----
The reference loss decomposes into:
  - 9 NT-Xent contrastive terms. For pair (A, B) with row-normalized
    embeddings Z = [An; Bn] (N=4096 rows, D=512), the per-row sim matrix is
    sim = (Zn @ Zn.T)/0.2 = 5*cos.  Since rows are unit-norm, sim[i,i] = 5.0
    is the exact row max, so
        lse_i (diag excluded) = 5 + log(S_i - 1),  S_i = sum_j exp(5*cos_ij - 5)
    and sum_i pos_i = 10 * sum_i cos(An_i, Bn_i).
    per-pair loss = 5 + (1/4096) sum_i log(S_i - 1) - (10/4096) sum_i cos_i
  - 12 cosine-embedding terms: 1 - (1/2048) sum_i cos_i.
  Total constant: 9*5 + 12 = 57.

Sharding (symmetric / block-cyclic)
-----------------------------------
The Gram matrix is symmetric: exp(sim)[i, j] contributes to S_i (row) AND
S_j (column).  Each 2048-row half is split into 8 blocks of 256; core c's
local block 0 is global block c (inputs are rolled by -256c rows, so the
SPMD program is identical on every core).  Per pair (A, B) each core
computes, with lhsT = its own 256 rows of A and B:

  A-rows x A-cols [0:1280): diag block [0:256) full (row-sums only),
      blocks d=1..3 [256:1024) with row-sums AND exp-column-sums,
      block d=4 [1024:1280) row-only (its mirror is computed by core c+4);
  A-rows x B-cols [0:2048): all with row+column sums (covers all cross
      pairs exactly once over the 8 cores);
  B-rows x B-cols [0:1280): same structure as A x A.

Column sums of exp are computed on the PE (selector-column ones matrix
against the bf16 exp tiles, accumulated in PSUM); per-row S is therefore
distributed across cores and assembled on the HOST, which applies
log(S - 1) and the final reduction (cheap: 9 x 4096 values).

Distributed row norms (this version)
------------------------------------
Every core still normalizes+transposes all 2048 rows of each tensor (the
Gram needs all columns), but the SUM-OF-SQUARES pass - previously ~100us
of DVE/ScalarE work per core, fully duplicated 8x - is distributed:
each core squares only its OWN 256 rows (24 x [128,512] ops), the
[128, 24] per-core sums are AllGather'd through DRAM, rotated into the
core's rolled frame with a gpsimd ap_gather (per-core int16 index input
carries the rotation - SPMD programs stay identical), and ln/exp then
produce all 192 rinv values in two ScalarE instructions.

Engine balance: ScalarE runs ONLY the Gram exps (+ the tiny rinv);
the PSUM->SBUF transpose drains are split DVE/Pool (with some ScalarE
only in the pre-exp f0/f1 window where it idles), and the 21 row-dot
reductions run on Pool.  Sim: ScalarE ~110us, PE ~103us, DVE ~95us,
Pool ~90us (baseline was ScalarE 198us / DVE 174us / Pool idle).

Normalization is folded into the PE transpose step as a regular matmul
x_tile.T @ diag(s * rinv) (the is_transpose matmul ignores the streamed
operand's values, so a true matmul is used).  Halves that are never a
pair's B side only need columns [0:1280) transposed/normalized.
"""

import numpy as np
import ml_dtypes
from contextlib import ExitStack

from concourse import bass, bacc, tile, mybir
from concourse.bass_utils import run_bass_kernel_spmd

BF16 = mybir.dt.bfloat16
FP8 = mybir.dt.float8e4
F32 = mybir.dt.float32
I16 = mybir.dt.int16
AF = mybir.ActivationFunctionType
ALU = mybir.AluOpType
DR = mybir.MatmulPerfMode.DoubleRow

# fp8 variant: Gram matmuls in fp8e4m3 with DoubleRow (2 MACs/cell/cycle).
# Normalized rows are pre-scaled by 16 so fp8 sees values ~N(0, 0.71^2);
# the Gram then yields 256*cos and the exp scale becomes 5/256.
USE_FP8 = True
FP8_SCALE = 16.0

B = 2048          # batch
DH = 512          # half feature dim
N_CORES = 8
R = B // N_CORES  # 256 rows per core shard
NT = B // 128     # 16 row tiles per half-matrix
KC = DH // 128    # 4 contraction chunks

ACOLS = 1280      # per-pair A-side column extent (superblocks d=0..4)
BCOLS = 2048      # per-pair B-side column extent
CS_W = 1024       # colsum psum row width
RT = ACOLS // 128  # build extent (tiles) for never-B halves

NAMES = ["f1_m0", "f1_m1", "f1_m2", "f2_m0", "f2_m1", "f2_m2"]

# contrastive pairs as ((f, h), (f, h)); h: 0 = shared, 1 = private.
# Pair index p: 0-2 = S1, 3-5 = private, 6-8 = S2 (matches dots cols 0-8).
PAIRS_S1 = [((0, 0), (1, 0)), ((0, 0), (2, 0)), ((1, 0), (2, 0))]
PAIRS_S2 = [((3, 0), (4, 0)), ((3, 0), (5, 0)), ((4, 0), (5, 0))]
PAIRS_P = [((0, 1), (3, 1)), ((1, 1), (4, 1)), ((2, 1), (5, 1))]
PAIRS_ALL = PAIRS_S1 + PAIRS_P + PAIRS_S2
ORTHO_V1 = [((0, 0), (0, 1)), ((1, 0), (1, 1)), ((2, 0), (2, 1)),
            ((0, 1), (1, 1)), ((0, 1), (2, 1)), ((1, 1), (2, 1))]
ORTHO_V2 = [((3, 0), (3, 1)), ((4, 0), (4, 1)), ((5, 0), (5, 1)),
            ((3, 1), (4, 1)), ((3, 1), (5, 1)), ((4, 1), (5, 1))]

# halves that are never the B side of any pair -> restricted build extent
RESTRICTED = {(0, 0), (3, 0), (0, 1), (1, 1), (2, 1)}

N_DOTS = 21       # 9 contrastive + 12 ortho row-dot sums

# norm-gather layout (f2-f5 only; f0/f1 norms are computed locally so
# their transposes need not wait out the collective's latency):
# contribution col = 4*(f-2) + 2*h + i (i = row-tile 0/1 within the
# core's 256-row block); rotated rinv col for local tile t of half
# (f, h) = 16*(t//2) + 4*(f-2) + 2*h + (t%2).
NRM_F0 = 2
NRM_C = 16
NRM_TOT = 8 * NRM_C


def _rinv_col(f, h, t):
    return NRM_C * (t // 2) + 4 * (f - NRM_F0) + 2 * h + (t % 2)


def build_program(use_fp8=USE_FP8, repeat=1, loads_on="sync",
                  copy_mod=2, cs_drain="vector",
                  dots_on="vector", ileave=(1, 2), pool_frac=(2, 13),
                  presq_split=2):
    # Restrict ACT table selection to the one set containing exp, ln AND
    # square (greedy per-op selection would otherwise thrash table sets,
    # ~1.3us per reload, serialized on ScalarE).
    if not getattr(bacc, "_ant_act_tables_patched", False):
        _orig_tables = bacc.get_activation_tables

        def _patched(arch):
            tabs = _orig_tables(arch)
            return {k: (v if k == "natural_log_exp_and_others" else set())
                    for k, v in tabs.items()}

        bacc.get_activation_tables = _patched
        bacc._ant_act_tables_patched = True

    nc = bacc.Bacc(
        "TRN2",
        target_bir_lowering=False,
        debug=False,
        enable_asserts=False,
        num_devices=N_CORES,
    )
    ffs = [nc.dram_tensor(n, [B, 2 * DH], BF16, kind="ExternalInput").ap()
           for n in NAMES]
    rotidx_dram = nc.dram_tensor("rotidx", [128, NRM_TOT // 16], I16,
                                 kind="ExternalInput").ap()
    rs_dram = nc.dram_tensor("rs", [128, 36], F32, kind="ExternalOutput").ap()
    dots_dram = nc.dram_tensor("dots", [128, 32], F32,
                               kind="ExternalOutput").ap()
    cs_dram = nc.dram_tensor("cs", [9, 8, 512], F32,
                             kind="ExternalOutput").ap()
    nrm_contrib = nc.dram_tensor("nrm_contrib", [128, NRM_C], F32,
                                 kind="Internal").ap()
    nrm_gath = nc.dram_tensor("nrm_gath", [N_CORES, 128, NRM_C], F32,
                              kind="Internal", addr_space="Shared").ap()

    ZDT = FP8 if use_fp8 else BF16
    exp_scale = 5.0 / (FP8_SCALE * FP8_SCALE) if use_fp8 else 5.0

    with tile.TileContext(nc) as tc, ExitStack() as ctx:
        znt_pool = ctx.enter_context(tc.tile_pool(name="zntp", bufs=8))
        x_pool = ctx.enter_context(tc.tile_pool(name="xp", bufs=6))
        dg_pool = ctx.enter_context(tc.tile_pool(name="dgp", bufs=4))
        vscr_pool = ctx.enter_context(tc.tile_pool(name="vscrp", bufs=3))
        escr_pool = ctx.enter_context(tc.tile_pool(name="escrp", bufs=3))
        sab_pool = ctx.enter_context(tc.tile_pool(name="sabp", bufs=4))
        csb_pool = ctx.enter_context(tc.tile_pool(name="csbp", bufs=2))
        acc_pool = ctx.enter_context(tc.tile_pool(name="accp", bufs=1))
        psum_pool = ctx.enter_context(
            tc.tile_pool(name="psump", bufs=2, space="PSUM"))

        load_eng = {"gpsimd": nc.gpsimd, "scalar": nc.scalar,
                    "sync": nc.sync}[loads_on]

        biasm5 = acc_pool.tile([128, 1], F32, tag="biasm5", name="biasm5")
        nc.gpsimd.memset(biasm5[:], -5.0)
        # bias for rinv: exp(-0.5 ln(ss) + ln(s)) = s/sqrt(ss)
        lnsc = acc_pool.tile([128, 1], F32, tag="lnsc", name="lnsc")
        nc.gpsimd.memset(lnsc[:], float(np.log(FP8_SCALE)) if use_fp8 else 0.0)
        # identity for the normalize-transpose: ident[p, j] = (j == p)
        ident = acc_pool.tile([128, 128], BF16, tag="ident", name="ident")
        iota_r = acc_pool.tile([128, 128], F32, tag="iota_r", name="iota_r")
        iota_p = acc_pool.tile([128, 1], F32, tag="iota_p", name="iota_p")
        nc.gpsimd.iota(iota_r[:], pattern=[[1, 128]], base=0,
                       channel_multiplier=0,
                       allow_small_or_imprecise_dtypes=True)
        nc.gpsimd.iota(iota_p[:], pattern=[[0, 1]], base=0,
                       channel_multiplier=1,
                       allow_small_or_imprecise_dtypes=True)
        nc.vector.tensor_scalar(
            out=ident[:], in0=iota_r[:], scalar1=iota_p[:, 0:1],
            scalar2=None, op0=ALU.is_equal)
        # selector-ones for exp colsums: onesel[p, m, r] = (m == r), so
        # lhsT = onesel[:, :, r] puts the colsum in psum partition r and
        # zeros (accumulate no-ops) in the other three.
        onesel = acc_pool.tile([128, 8, 8], BF16, tag="onesel", name="onesel")
        io_m = acc_pool.tile([128, 8, 8], F32, tag="io_m", name="io_m")
        io_r = acc_pool.tile([128, 8, 8], F32, tag="io_r", name="io_r")
        nc.gpsimd.iota(io_m[:], pattern=[[1, 8], [0, 8]], base=0,
                       channel_multiplier=0,
                       allow_small_or_imprecise_dtypes=True)
        nc.gpsimd.iota(io_r[:], pattern=[[0, 8], [1, 8]], base=0,
                       channel_multiplier=0,
                       allow_small_or_imprecise_dtypes=True)
        nc.vector.tensor_tensor(out=onesel[:], in0=io_m[:], in1=io_r[:],
                                op=ALU.is_equal)
        # zero selector for colsum-psum bracketing matmuls
        zsel = acc_pool.tile([128, 8], ZDT, tag="zsel", name="zsel")
        nc.gpsimd.memset(zsel[:], 0.0)

        cp_i = [0]
        csd_i = [0]
        rs = acc_pool.tile([128, 36], F32, tag="rs", name="rs_sb")
        dots_all = acc_pool.tile([128, 32], F32, tag="dots", name="dots_all")
        nc.vector.memset(dots_all[:], 0.0)

        znt = {}

        rep_ctx = tc.For_i(0, repeat, 1) if repeat > 1 else None
        if rep_ctx is not None:
            rep_ctx.__enter__()

        # ---- distributed row-norm prologue -------------------------------
        # Square ONLY this core's own 256 rows (rolled rows [0:256)) of each
        # tensor half, AllGather the per-core [128, 24] sums via DRAM, and
        # rotate the [128, 8, 24] result into this core's rolled frame with
        # ap_gather (the per-core rotation lives in the rotidx input data).
        rotidx = acc_pool.tile([128, NRM_TOT // 16], I16, tag="rotidx",
                               name="rotidx")
        load_eng.dma_start(out=rotidx[:], in_=rotidx_dram)
        nrm_own = acc_pool.tile([128, NRM_C], F32, tag="nrm_own",
                                name="nrm_own")
        own_xt = {}
        for f in range(NRM_F0, 6):
            # own tag: these stay alive until build_ff(f) consumes them
            xo = x_pool.tile([128, 2, 2 * DH], BF16, tag="xto", bufs=4,
                             name=f"xto{f}")
            load_eng.dma_start(
                out=xo[:],
                in_=ffs[f][0:256, :].rearrange("(tt p) c -> p tt c", p=128))
            own_xt[f] = xo
        sq_i = 0
        for f in range(NRM_F0, 6):
            for h in range(2):
                for i in range(2):
                    xv = own_xt[f][:, i, h * DH:(h + 1) * DH]
                    col = 4 * (f - NRM_F0) + 2 * h + i
                    sq = vscr_pool.tile([128, DH], BF16, tag="psq", bufs=6,
                                        name=f"psq{f}_{h}_{i}")
                    # split the 24 prologue squares across DVE/ScalarE
                    # (Pool's ISA rejects scalar_tensor_tensor) so the
                    # contribution DMA (and the collective behind it) fires
                    # as early as possible
                    eng = nc.scalar if sq_i % 2 == 1 else nc.vector
                    if eng is nc.scalar:
                        nc.scalar.activation(
                            sq[:], xv, AF.Square,
                            accum_out=nrm_own[:, col:col + 1])
                    else:
                        eng.scalar_tensor_tensor(
                            out=sq[:], in0=xv, scalar=1.0, in1=xv,
                            op0=ALU.mult, op1=ALU.mult,
                            accum_out=nrm_own[:, col:col + 1])
                    sq_i += 1
        nc.sync.dma_start(out=nrm_contrib, in_=nrm_own[:])
        nc.gpsimd.collective_compute(
            kind="AllGather", op=ALU.bypass,
            replica_groups=[list(range(N_CORES))],
            ins=[nrm_contrib], outs=[nrm_gath],
        )
        nrm_glob = acc_pool.tile([128, N_CORES, NRM_C], F32, tag="nrm_glob",
                                 name="nrm_glob")
        nc.sync.dma_start(out=nrm_glob[:],
                          in_=nrm_gath.rearrange("s p c -> p s c"))
        nrm_rot = acc_pool.tile([128, NRM_TOT], F32, tag="nrm_rot",
                                name="nrm_rot")
        nc.gpsimd.ap_gather(
            out_ap=nrm_rot[:].rearrange("p (n d) -> p n d", d=1),
            in_ap=nrm_glob[:].rearrange("p b c -> p (b c)").rearrange(
                "p (n d) -> p n d", d=1),
            idxs_ap=rotidx[:],
            channels=128, num_elems=NRM_TOT, d=1, num_idxs=NRM_TOT,
        )
        lgn_all = acc_pool.tile([128, NRM_TOT], F32, tag="lgn_all",
                                name="lgn_all")
        rinv_all = acc_pool.tile([128, NRM_TOT], F32, tag="rinv_all",
                                 name="rinv_all")
        nc.scalar.activation(lgn_all[:], nrm_rot[:], AF.Ln)
        nc.scalar.activation(rinv_all[:], lgn_all[:], AF.Exp,
                             scale=-0.5, bias=lnsc[:])

        def build_ff(f):
            """Load ff tensor f; per half, write the normalized transpose
            znt[(f,h)][p, c, j] = s*Zn_h[j, c*128 + p] via PE matmul
            x_tile.T @ diag(s*rinv).  rinv comes from the gathered+rotated
            rinv_all tile.  Never-B halves only build columns [0:ACOLS).

            Generator: yields once per g-group prologue and once per
            transposed tile, so the driver can interleave build emission
            with gram emission (ScalarE-heavy)."""
            ext = {h: (RT if (f, h) in RESTRICTED else NT) for h in range(2)}
            max_t = max(ext.values())
            zts = []
            for h in range(2):
                cols = ext[h] * 128
                zts.append(znt_pool.tile(
                    [128, KC, cols], ZDT,
                    tag=("znt_r" if ext[h] == RT else "znt_f"),
                    bufs=(5 if ext[h] == RT else 8),
                    name=f"znt{f}_{h}"))
                znt[(f, h)] = zts[h]
            local_nrm = f < NRM_F0
            if local_nrm:
                # f0/f1: norms computed locally tile-by-tile (DVE+ScalarE
                # both idle pre-exp), so transposes need not wait for the
                # collective.  norms interleaved (t, h) -> col 2t+h.
                norms = acc_pool.tile([128, 2 * NT], F32, tag=f"nrm{f}",
                                      name=f"nrm{f}")
                lgn = acc_pool.tile([128, 2 * NT], F32, tag=f"lgn{f}",
                                    name=f"lgn{f}")
                rinv = acc_pool.tile([128, 2 * NT], F32, tag=f"rinv{f}",
                                     name=f"rinv{f}")
            for g in range((max_t + 3) // 4):
                ts = list(range(4 * g, min(4 * g + 4, max_t)))
                xts = []
                for u in range(2):
                    base = (4 * g + 2 * u) * 128
                    if base >= max_t * 128:
                        xts.append(None)
                        continue
                    if g == 0 and u == 0 and f in own_xt:
                        # tiles 0-1 were already loaded by the norm prologue
                        xts.append(own_xt[f])
                        continue
                    # one 3D DMA covers two 128-row tiles: [128, 2, 1024]
                    xt = x_pool.tile([128, 2, 2 * DH], BF16, tag="xt",
                                     name=f"xt{f}_{g}_{u}")
                    load_eng.dma_start(
                        out=xt[:],
                        in_=ffs[f][base:base + 256, :].rearrange(
                            "(tt p) c -> p tt c", p=128))
                    xts.append(xt)
                if local_nrm:
                    for i, t in enumerate(ts):
                        xv = xts[i // 2][:, i % 2, :]
                        for h in range(2):
                            if t >= ext[h]:
                                continue
                            sq = vscr_pool.tile([128, DH], BF16, tag="psq",
                                                bufs=6, name=f"sq{f}_{h}_{t}")
                            if (2 * t + h) % 2 == 1:
                                nc.scalar.activation(
                                    sq[:], xv[:, h * DH:(h + 1) * DH],
                                    AF.Square,
                                    accum_out=norms[:, 2 * t + h:2 * t + h + 1])
                            else:
                                nc.vector.scalar_tensor_tensor(
                                    out=sq[:], in0=xv[:, h * DH:(h + 1) * DH],
                                    scalar=1.0,
                                    in1=xv[:, h * DH:(h + 1) * DH],
                                    op0=ALU.mult, op1=ALU.mult,
                                    accum_out=norms[:, 2 * t + h:2 * t + h + 1])
                    # rinv = s/sqrt(ss) over the active (t, h) cols
                    runs = []
                    if all(t < ext[h] for t in ts for h in range(2)):
                        runs.append((2 * ts[0], len(ts) * 2, 1))
                    else:
                        for h in range(2):
                            hts = [t for t in ts if t < ext[h]]
                            if hts:
                                runs.append((2 * hts[0] + h, len(hts), 2))
                    for (c0, n, st) in runs:
                        sl = (slice(c0, c0 + st * (n - 1) + 1, st) if st > 1
                              else slice(c0, c0 + n))
                        nc.scalar.activation(lgn[:, sl], norms[:, sl], AF.Ln)
                        nc.scalar.activation(rinv[:, sl], lgn[:, sl], AF.Exp,
                                             scale=-0.5, bias=lnsc[:])
                yield
                for i, t in enumerate(ts):
                    xv = xts[i // 2][:, i % 2, :]
                    for h in range(2):
                        if t >= ext[h]:
                            continue
                        dg = dg_pool.tile([128, 128], BF16, tag="dg",
                                          bufs=6, name=f"dg{f}_{h}_{t}")
                        if local_nrm:
                            # DVE: Pool's in-order queue is parked behind
                            # the ap_gather waiting on the collective
                            rsrc = rinv[:, 2 * t + h:2 * t + h + 1]
                            nc.vector.tensor_scalar_mul(
                                out=dg[:], in0=ident[:], scalar1=rsrc)
                        else:
                            rc = _rinv_col(f, h, t)
                            # Pool (sbuf-only ops are legal there)
                            nc.gpsimd.tensor_scalar_mul(
                                out=dg[:], in0=ident[:],
                                scalar1=rinv_all[:, rc:rc + 1])
                        tp = psum_pool.tile([128, KC, 128], F32,
                                            tag="tpp", bufs=2,
                                            name=f"tp{f}_{h}_{t}")
                        for c in range(KC):
                            nc.tensor.matmul(
                                tp[:, c, :],
                                xv[:, h * DH + c * 128:h * DH + (c + 1) * 128],
                                dg[:], start=True, stop=True)
                        dst = zts[h][:, :, t * 128:(t + 1) * 128]
                        # PSUM drains can only run on DVE or ScalarE (Pool
                        # has no PSUM access).  ScalarE helps only in the
                        # pre-exp f0/f1 window where it would idle;
                        # afterwards every pool_frac-th drain goes ScalarE
                        # (default 0 = none, ScalarE is exp-bound).
                        pnum, pden = pool_frac
                        if f < 2:
                            on_scalar = cp_i[0] % 2 == 1
                        else:
                            on_scalar = pnum and (cp_i[0] * pnum) % pden < pnum
                        if on_scalar:
                            nc.scalar.copy(dst, tp[:, :, :])
                        else:
                            nc.vector.tensor_copy(dst, tp[:, :, :])
                        cp_i[0] += 1
                        yield

        def gblock(ps_ap, X, mt, RH, c0, c1):
            """DoubleRow (or bf16) matmul group: Gram rows of X's mtile mt
            against RH columns [c0:c1)."""
            lt = znt[X]
            rt = znt[RH]
            if use_fp8:
                for q in range(KC // 2):
                    nc.tensor.matmul(
                        ps_ap,
                        lt[:, 2 * q:2 * q + 2, mt * 128:(mt + 1) * 128],
                        rt[:, 2 * q:2 * q + 2, c0:c1],
                        perf_mode=DR, start=(q == 0), stop=(q == KC // 2 - 1))
            else:
                for kc in range(KC):
                    nc.tensor.matmul(
                        ps_ap, lt[:, kc, mt * 128:(mt + 1) * 128],
                        rt[:, kc, c0:c1],
                        start=(kc == 0), stop=(kc == KC - 1))

        def gram_mm(p, A, Bm):
            """Symmetric-scheme Gram matmuls + exp row-sums for pair p.
            Returns the es tiles needed by the deferred colsum pass
            (gram_cs) - deferring keeps colsum matmuls, which wait on
            ScalarE exps, from stalling the in-order PE queue between
            consecutive gram fills."""
            sab = sab_pool.tile([128, 12], F32, tag="sab", name=f"sab{p}")

            def exp_block(ps, width, col, name):
                es = escr_pool.tile([128, 2, width], BF16, tag="escr",
                                    bufs=17, name=name)
                nc.scalar.activation(es[:], ps[:], AF.Exp, bias=biasm5[:],
                                     scale=exp_scale,
                                     accum_out=sab[:, col:col + 1])
                return es

            es = {}
            # ---- A-rows ----
            for mt in range(2):
                # T0: A[0:1024): diag [0:256) row-only + d=1..3 first part
                g0 = psum_pool.tile([128, 2, 512], F32, tag="gram", bufs=2,
                                    name=f"g0_{p}_{mt}")
                for cb in range(2):
                    gblock(g0[:, cb, :], A, mt, A, cb * 512, (cb + 1) * 512)
                es[(0, mt)] = exp_block(g0, 512, 4 * mt + 0, f"es0_{p}_{mt}")
                yield
                # T2: B[0:1024) cross, full row+col
                g2 = psum_pool.tile([128, 2, 512], F32, tag="gram", bufs=2,
                                    name=f"g2_{p}_{mt}")
                for cb in range(2):
                    gblock(g2[:, cb, :], A, mt, Bm, cb * 512, (cb + 1) * 512)
                es[(2, mt)] = exp_block(g2, 512, 4 * mt + 1, f"es2_{p}_{mt}")
                yield
                # T3: B[1024:2048) cross
                g3 = psum_pool.tile([128, 2, 512], F32, tag="gram", bufs=2,
                                    name=f"g3_{p}_{mt}")
                for cb in range(2):
                    gblock(g3[:, cb, :], A, mt, Bm,
                           1024 + cb * 512, 1024 + (cb + 1) * 512)
                es[(3, mt)] = exp_block(g3, 512, 4 * mt + 2, f"es3_{p}_{mt}")
                yield
            # T1 tails: A[1024:1280) row-only (d=4, mirrored by core c+4);
            # both mtiles share one 1-bank psum tile
            gt = psum_pool.tile([128, 2, 256], F32, tag="tail", bufs=1,
                                name=f"gt_{p}")
            for mt in range(2):
                gblock(gt[:, mt, :], A, mt, A, 1024, 1280)
            for mt in range(2):
                est = escr_pool.tile([128, 256], BF16, tag="escr_t", bufs=8,
                                     name=f"est_{p}_{mt}")
                nc.scalar.activation(est[:], gt[:, mt, :], AF.Exp,
                                     bias=biasm5[:], scale=exp_scale,
                                     accum_out=sab[:, 4 * mt + 3:4 * mt + 4])
            yield
            # ---- B-rows ----
            for mt in range(2):
                # T4: B[0:1024): diag row-only + d=1..3 colsum part
                g4 = psum_pool.tile([128, 2, 512], F32, tag="gram", bufs=2,
                                    name=f"g4_{p}_{mt}")
                for cb in range(2):
                    gblock(g4[:, cb, :], Bm, mt, Bm, cb * 512, (cb + 1) * 512)
                es[(4, mt)] = exp_block(g4, 512, 8 + 2 * mt, f"es4_{p}_{mt}")
                yield
            # T5 tails: B[1024:1280) row-only
            gu = psum_pool.tile([128, 2, 256], F32, tag="tail", bufs=1,
                                name=f"gu_{p}")
            for mt in range(2):
                gblock(gu[:, mt, :], Bm, mt, Bm, 1024, 1280)
            for mt in range(2):
                esu = escr_pool.tile([128, 256], BF16, tag="escr_t", bufs=8,
                                     name=f"esu_{p}_{mt}")
                nc.scalar.activation(esu[:], gu[:, mt, :], AF.Exp,
                                     bias=biasm5[:], scale=exp_scale,
                                     accum_out=sab[:, 9 + 2 * mt:10 + 2 * mt])
            cs_states[p] = (p, A, sab, es)

        cs_states = {}

        def gram_cs(p_):
            """Deferred colsum pass for a pair whose exps have long
            completed: bracket-open, 12 selector-ones matmuls, bracket-
            close, rowsum reductions, colsum drain + DMA."""
            p, A, sab, es = cs_states.pop(p_)
            cs = psum_pool.tile([8, 512], F32, tag="cs", bufs=1,
                                name=f"cs{p}")
            # Constraints: moving APs must be 2D (s3d3_mm_num_elements) and
            # a matmul's psum output cannot cross a bank (<=512 f32 wide).
            # Rows: 0 = A[256:768), 1 = A[768:1024), 2..5 = B quarters.
            dummy = znt[A][:, 0, 0:512]
            # bracket-open: zero the whole colsum region (sets has_written)
            nc.tensor.matmul(cs[0:8, :], zsel[:], dummy, start=True,
                             stop=False)

            def csum(es_ap, r, off, w):
                nc.tensor.matmul(cs[0:8, off:off + w], onesel[:, :, r],
                                 es_ap, start=False, stop=False)

            for mt in range(2):
                csum(es[(0, mt)][:, 0, 256:512], 0, 0, 256)
                csum(es[(0, mt)][:, 1, 0:256], 0, 256, 256)
                csum(es[(0, mt)][:, 1, 256:512], 1, 0, 256)
                for cb in range(2):
                    csum(es[(2, mt)][:, cb, :], 2 + cb, 0, 512)
                    csum(es[(3, mt)][:, cb, :], 4 + cb, 0, 512)
                csum(es[(4, mt)][:, 0, 256:512], 2, 256, 256)
                csum(es[(4, mt)][:, 1, :], 3, 0, 512)
            # bracket-close
            nc.tensor.matmul(cs[0:8, :], zsel[:], dummy, start=False,
                             stop=True)
            # per-mtile row-sum reduction into rs
            for mtidx, (c0, c1) in enumerate([(0, 4), (4, 8), (8, 10),
                                              (10, 12)]):
                scr = sab_pool.tile([128, 4], F32, tag="scr2",
                                    name=f"scr_{p}_{mtidx}")
                nc.vector.tensor_scalar(
                    out=scr[:, 0:c1 - c0], in0=sab[:, c0:c1], scalar1=0.0,
                    scalar2=None, op0=ALU.add, op1=ALU.add,
                    accum_out=rs[:, 4 * p + mtidx:4 * p + mtidx + 1])
            # drain colsums -> SBUF -> DRAM
            csb = csb_pool.tile([8, 512], F32, tag="csb", name=f"csb{p}")
            if cs_drain == "scalar" or (cs_drain == "alt" and csd_i[0] % 2):
                nc.scalar.copy(csb[:], cs[:])
            else:
                nc.vector.tensor_copy(csb[:], cs[:])
            csd_i[0] += 1
            nc.sync.dma_start(out=cs_dram[p], in_=csb[:])

        def dots(col, X, Y):
            """dots_all[:, col] = per-partition sum over the core 256-row
            shard of <Zn_X[i], Zn_Y[i]> (row-wise cosines)."""
            o = vscr_pool.tile([128, KC, R], F32, tag="vscr", name=f"do{col}")
            dscale = 1.0 / (FP8_SCALE * FP8_SCALE) if use_fp8 else 1.0
            d_eng = nc.gpsimd if dots_on == "gpsimd" else nc.vector
            d_eng.scalar_tensor_tensor(
                out=o[:], in0=znt[X][:, :, 0:R], scalar=dscale,
                in1=znt[Y][:, :, 0:R], op0=ALU.mult, op1=ALU.mult,
                accum_out=dots_all[:, col:col + 1])

        # Interleaved emission: builds are DVE/Pool-heavy (diags, drains),
        # grams ScalarE-heavy (exps).  A phase-structured emission leaves
        # each engine idle half the time, so the driver round-robins one
        # build-tile step against one gram step, gating each gram-side item
        # on its tensors' build emission being complete.  dots_all cols:
        # 0..8 contrastive (s1 x3, private x3, s2 x3); 9..20 ortho.
        def one_shot(fn, *a):
            def g():
                fn(*a)
                yield
            return g

        # (prereq builds emitted, generator factory)
        gram_items = [
            ({0, 1}, lambda: gram_mm(0, (0, 0), (1, 0))),
            ({0, 1}, one_shot(dots, 0, (0, 0), (1, 0))),
            ({0, 1}, one_shot(dots, 9, (0, 0), (0, 1))),
            ({0, 1}, one_shot(dots, 10, (1, 0), (1, 1))),
            ({0, 1}, one_shot(dots, 12, (0, 1), (1, 1))),
            ({2}, lambda: gram_mm(1, (0, 0), (2, 0))),
            ({2}, one_shot(gram_cs, 0)),
            ({2}, lambda: gram_mm(2, (1, 0), (2, 0))),
            ({2}, one_shot(dots, 1, (0, 0), (2, 0))),
            ({2}, one_shot(dots, 2, (1, 0), (2, 0))),
            ({2}, one_shot(dots, 11, (2, 0), (2, 1))),
            ({2}, one_shot(dots, 13, (0, 1), (2, 1))),
            ({2}, one_shot(dots, 14, (1, 1), (2, 1))),
            ({2}, one_shot(gram_cs, 1)),
            ({3}, lambda: gram_mm(3, (0, 1), (3, 1))),
            ({3}, one_shot(gram_cs, 2)),
            ({3}, one_shot(dots, 3, (0, 1), (3, 1))),
            ({3}, one_shot(dots, 15, (3, 0), (3, 1))),
            ({4}, lambda: gram_mm(4, (1, 1), (4, 1))),
            ({4}, one_shot(gram_cs, 3)),
            ({4}, lambda: gram_mm(6, (3, 0), (4, 0))),
            ({4}, one_shot(gram_cs, 4)),
            ({4}, one_shot(dots, 4, (1, 1), (4, 1))),
            ({4}, one_shot(dots, 6, (3, 0), (4, 0))),
            ({4}, one_shot(dots, 16, (4, 0), (4, 1))),
            ({4}, one_shot(dots, 18, (3, 1), (4, 1))),
            ({5}, lambda: gram_mm(5, (2, 1), (5, 1))),
            ({5}, one_shot(gram_cs, 6)),
            ({5}, lambda: gram_mm(7, (3, 0), (5, 0))),
            ({5}, one_shot(gram_cs, 5)),
            ({5}, lambda: gram_mm(8, (4, 0), (5, 0))),
            ({5}, one_shot(gram_cs, 7)),
            ({5}, one_shot(dots, 5, (2, 1), (5, 1))),
            ({5}, one_shot(dots, 7, (3, 0), (5, 0))),
            ({5}, one_shot(dots, 8, (4, 0), (5, 0))),
            ({5}, one_shot(dots, 17, (5, 0), (5, 1))),
            ({5}, one_shot(dots, 19, (3, 1), (5, 1))),
            ({5}, one_shot(dots, 20, (4, 1), (5, 1))),
            ({5}, one_shot(gram_cs, 8)),
        ]

        built = set()

        def chain_builds():
            for f in range(6):
                yield from build_ff(f)
                built.add(f)

        # builds f0+f1 first (nothing to overlap), then round-robin
        bgen = chain_builds()
        gq = list(gram_items)
        cur = None
        bdone = False
        while not bdone and not built >= {0, 1}:
            try:
                next(bgen)
            except StopIteration:
                bdone = True
        gs_per, bs_per = ileave
        while cur is not None or gq or not bdone:
            stepped_gram = False
            steps = 0
            while steps < gs_per:
                if cur is None and gq and gq[0][0] <= built:
                    cur = gq.pop(0)[1]()
                if cur is None:
                    break
                try:
                    next(cur)
                    stepped_gram = True
                    steps += 1
                except StopIteration:
                    cur = None
            if not bdone:
                for _ in range(bs_per):
                    try:
                        next(bgen)
                    except StopIteration:
                        bdone = True
                        break
            elif not stepped_gram and cur is None and gq:
                # builds done but queue head unexpectedly gated
                raise RuntimeError("emission driver stalled")

        # ---- epilogue: ship raw partial sums; host assembles S ----
        nc.sync.dma_start(out=rs_dram, in_=rs[:])
        nc.sync.dma_start(out=dots_dram, in_=dots_all[:])

        if rep_ctx is not None:
            rep_ctx.__exit__(None, None, None)

    nc.compile()
    return nc


_PROG = None


def _get_prog():
    global _PROG
    if _PROG is None:
        _PROG = build_program()
    return _PROG


def _make_rotidx(c):
    """ap_gather indices rotating the gathered [8, 24] norm blocks into
    core c's rolled frame: flat out elem j = d*24+col <- (c+d)%8*24+col.
    Packing: index j lives at partition j%16, slot j//16 (replicated
    across all eight 16-partition groups)."""
    want = np.array([((c + (j // NRM_C)) % N_CORES) * NRM_C + (j % NRM_C)
                     for j in range(NRM_TOT)], dtype=np.int16)
    idx = np.zeros((128, NRM_TOT // 16), dtype=np.int16)
    for p in range(128):
        for k in range(NRM_TOT // 16):
            idx[p, k] = want[k * 16 + (p % 16)]
    return idx


def make_in_maps(inputs):
    bf = ml_dtypes.bfloat16
    in_maps = []
    for c in range(N_CORES):
        m = {}
        for n in NAMES:
            a = np.asarray(inputs[n], dtype=np.float32)
            m[n] = np.ascontiguousarray(np.roll(a, -R * c, axis=0)).astype(bf)
        m["rotidx"] = _make_rotidx(c)
        in_maps.append(m)
    return in_maps


def combine(results):
    """results: list of 8 dicts with 'rs' [128,36], 'dots' [128,32],
    'cs' [9,4,1024] -> scalar loss.  Assembles the distributed exp-sums S
    per pair (A rows 0:2048, B rows 2048:4096) and applies log(S-1)."""
    S = np.zeros((9, 2, B), dtype=np.float64)
    tcc = toc = 0.0
    for c in range(N_CORES):
        rs = np.asarray(results[c]["rs"], dtype=np.float64)
        cso = np.asarray(results[c]["cs"], dtype=np.float64)
        dots = np.asarray(results[c]["dots"], dtype=np.float64)
        tcc += dots[:, 0:9].sum()
        toc += dots[:, 9:21].sum()
        sh = R * c
        for p in range(9):
            for mt in range(4):
                half = 0 if mt < 2 else 1
                base = sh + (mt % 2) * 128
                idx = (base + np.arange(128)) % B
                S[p, half, idx] += rs[:, 4 * p + mt]
            # cs rows: 0 = A[256:768), 1[0:256] = A[768:1024),
            # 2..5 = B[0:512), [512:1024), [1024:1536), [1536:2048)
            idxA = (sh + 256 + np.arange(512)) % B
            S[p, 0, idxA] += cso[p, 0]
            idxA2 = (sh + 768 + np.arange(256)) % B
            S[p, 0, idxA2] += cso[p, 1, 0:256]
            for q in range(4):
                idxB = (sh + 512 * q + np.arange(512)) % B
                S[p, 1, idxB] += cso[p, 2 + q]
    tl = np.log(np.maximum(S - 1.0, 1e-30)).sum()
    n2 = float(2 * B)
    loss = (9 * 5.0 + 12.0) + tl / n2 - 10.0 * tcc / n2 - toc / float(B)
    return np.float32(loss)


def kernel(**inputs):
    nc = _get_prog()
    in_maps = make_in_maps(inputs)
    res = run_bass_kernel_spmd(nc, in_maps, list(range(N_CORES)))
    return combine(res.results)
